# revision 15
# baseline (speedup 1.0000x reference)
"""Trainium2 Bass kernel for nn_NodeEmbeddingLayer (MoE-routed node embedding).

Reference math (the dense [N,E,D] all-expert einsum collapses to this because
the output only reads the G=256 rows selected by g_node_id):

    xm    = node_fea[g] * masks[g]                  # [G, F]
    emb   = xm @ W_emb[node_type[g]] + b_emb[...]   # routed per-node expert
    h     = emb @ W_hid + b_hid                     # [G, H]
    probs = softmax(h @ W_out + b_out)              # [G, 2]  (2 classes)
    loss  = -sum(log_softmax(probs)[g, label[g]])

The network is linear from the routed expert layer to the softmax (eval-mode
dropout is identity), and a 2-class softmax/CE depends only on the logit
difference d = l0 - l1. So the whole weight chain folds per expert (in fp64,
on host, as weight preprocessing — standard linear-layer fusion):

    wfold[e] = W_emb[e] @ W_hid @ (W_out[:,0] - W_out[:,1])      # [F]
    bias[e]  = (b_emb[e] @ W_hid + b_hid) @ (W_out[:,0]-W_out[:,1])
               + b_out[0] - b_out[1]

    d[n]  = xm[n] . wfold[type[n]] + bias[type[n]]
    p0    = sigmoid(d) = 1/(1+exp(-d));  p1 = exp(-d) * p0
    nll   = log(e^{p0} + e^{p1}) - probs[label]   (log_softmax OF probs)
    loss  = sum over nodes of nll

Sharding: nodes are grouped by node_type, each group is chunked into slices
of <=32 nodes (a PSUM base-partition quantum), and the slices are dealt
S = ceil(total/8) per core (S <= 3 for any distribution of 256 nodes over 16
types) so all 8 cores run one SPMD NEFF. Everything a core needs rides in one
packed [128, ~200] f32 DMA. On device: 2 DVE muls (xm), 4 routed-dot matmuls into PSUM, an
Exp/Ln-only tail (single activation-table load, warmed during the DMA), a
valid-masked matmul loss reduction, and one merged output DMA (probs+loss).
"""

import sys

for _p in ("/opt/trn_rl_repo", "/root/.axon_site/_ro/trn_rl_repo"):
    if _p not in sys.path:
        sys.path.append(_p)

import numpy as np

import concourse.mybir as mybir
import concourse.tile as tile
from concourse import bacc
from concourse.bass_utils import run_bass_kernel_spmd

# Prefer the activation table holding BOTH Exp and Ln so the whole kernel
# needs a single LoadActFuncSet (the default order splits them across two
# tables, putting a 1.3us table load on the critical path before Ln).
import concourse.bacc as _bacc_mod
from concourse.hw_specs import get_activation_tables as _gat


def _gat_only_lnexp(arch):
    # Keep dict order (act_func_set_id is positional) but blank the other
    # tables so the chooser can only pick the combined Exp+Ln table.
    tabs = _gat(arch)
    pref = "natural_log_exp_and_others"
    if pref not in tabs:
        return tabs
    return {k: (v if k == pref else type(v)()) for k, v in tabs.items()}


_bacc_mod.get_activation_tables = _gat_only_lnexp

F, E = 256, 16
G = 256
NCORES = 8
P = 128
KF = F // P  # contraction tiles over the feature dim
SL = 32      # nodes per slice (PSUM base-partition quantum)

LAST_RESULTS = None  # BassKernelResults of the most recent run (for profiling)


def _offsets(S):
    W2 = S * SL
    c_xT = 0
    c_mT = c_xT + KF * W2
    c_wf = c_mT + KF * W2      # folded weight columns, one per (slice, ktile)
    c_nb = c_wf + S * KF       # -bias[type[n]] per node
    c_s1 = c_nb + 1            # 1 - 2*label[n]
    c_s2 = c_s1 + 1            # label[n]
    c_vd = c_s2 + 1            # 1.0 for real nodes, 0.0 for padding
    return W2, c_xT, c_mT, c_wf, c_nb, c_s1, c_s2, c_vd, c_vd + 1


def _build(S: int):
    """Build the SPMD NEFF for S slices of SL nodes per core."""
    nc = bacc.Bacc("TRN2", target_bir_lowering=False, debug=False,
                   num_devices=NCORES)
    f32 = mybir.dt.float32
    W2, c_xT, c_mT, c_wf, c_nb, c_s1, c_s2, c_vd, PKW = _offsets(S)

    pk = nc.dram_tensor("pk", [P, PKW], f32, kind="ExternalInput")
    out_t = nc.dram_tensor("out", [W2, 3], f32, kind="ExternalOutput")

    with tile.TileContext(nc) as tc:
        with (
            tc.tile_pool(name="work", bufs=1) as work,
            tc.tile_pool(name="psum", bufs=1, space="PSUM") as pp,
        ):
            # ACT table warmup (Exp+Ln live in one table) during the DMA
            wt = work.tile([1, 1], f32, tag="wt")
            nc.vector.memset(wt[:], 0.25)
            wt2 = work.tile([1, 1], f32, tag="wt2")
            nc.scalar.activation(wt2[:], wt[:],
                                 mybir.ActivationFunctionType.Exp)
            nll_pad = work.tile([P, 1], f32, tag="nll_pad")
            nc.vector.memset(nll_pad[:], 0.0)

            pk_sb = work.tile([P, PKW], f32, tag="pk")
            nc.sync.dma_start(pk_sb[:], pk[:])

            def pkc(c0):
                return pk_sb[:, c0 : c0 + 1]

            # xm = x * m  (node columns grouped per expert slot)
            xm = work.tile([P, KF, W2], f32, tag="xm")
            for k in range(KF):
                nc.vector.tensor_mul(
                    xm[:, k, :],
                    pk_sb[:, c_xT + k * W2 : c_xT + (k + 1) * W2],
                    pk_sb[:, c_mT + k * W2 : c_mT + (k + 1) * W2],
                )

            # routed dot: d[n] = xm[n] . wfold[slice(n)]
            ps_d = pp.tile([W2, 1], f32, tag="ps_d")
            for s in range(S):
                rows = slice(s * SL, (s + 1) * SL)
                for k in range(KF):
                    nc.tensor.matmul(
                        ps_d[rows, :],
                        xm[:, k, rows],
                        pkc(c_wf + s * KF + k),
                        start=(k == 0),
                        stop=(k == KF - 1),
                    )

            # probs: e0 = exp(-(d+bias)); p0 = 1/(1+e0); p1 = e0*p0
            e0 = work.tile([W2, 1], f32, tag="e0")
            nc.scalar.activation(e0[:], ps_d[:],
                                 mybir.ActivationFunctionType.Exp,
                                 bias=pk_sb[:W2, c_nb : c_nb + 1], scale=-1.0)
            s1 = work.tile([W2, 1], f32, tag="s1")
            nc.vector.tensor_scalar_add(s1[:], e0[:], 1.0)
            probs_t = work.tile([W2, 3], f32, tag="probs_t")
            nc.vector.reciprocal(probs_t[:, 0:1], s1[:])
            nc.vector.tensor_mul(probs_t[:, 1:2], e0[:], probs_t[:, 0:1])

            # nll = log(e^p0 + e^p1) - p_label  (log_softmax applied to probs)
            ee = work.tile([W2, 2], f32, tag="ee")
            esum = work.tile([W2, 1], f32, tag="esum")
            nc.scalar.activation(ee[:], probs_t[:, 0:2],
                                 mybir.ActivationFunctionType.Exp,
                                 accum_out=esum[:])
            lse = work.tile([W2, 1], f32, tag="lse")
            nc.scalar.activation(lse[:], esum[:],
                                 mybir.ActivationFunctionType.Ln, bias=0.0)
            # p_label = p0*(1-2*lab) + lab
            pl = work.tile([W2, 1], f32, tag="pl")
            nc.vector.tensor_scalar(pl[:], probs_t[:, 0:1],
                                    pk_sb[:W2, c_s1 : c_s1 + 1],
                                    pk_sb[:W2, c_s2 : c_s2 + 1],
                                    mybir.AluOpType.mult, mybir.AluOpType.add)
            nc.vector.tensor_sub(nll_pad[:W2, :], lse[:], pl[:])

            # partial loss = sum over real nodes (valid column masks padding)
            ps_loss = pp.tile([1, 1], f32, tag="ps_loss")
            nc.tensor.matmul(ps_loss[:], nll_pad[:],
                             pk_sb[:, c_vd : c_vd + 1],
                             start=True, stop=True)
            nc.vector.tensor_copy(probs_t[0:1, 2:3], ps_loss[:])

            nc.sync.dma_start(out_t[:], probs_t[:])

    nc.compile()
    return nc


def kernel(node_fea, masks, node_type, g_node_id, label,
           W_emb, b_emb, W_hid, b_hid, W_out, b_out):
    global LAST_RESULTS
    node_fea = np.asarray(node_fea, dtype=np.float32)
    masks = np.asarray(masks, dtype=np.float32)
    g = np.asarray(g_node_id).astype(np.int64)
    t_all = np.asarray(node_type).astype(np.int64)
    lab = np.asarray(label).astype(np.int64)

    # fold the all-linear head (fp64): per-expert readout vector + bias
    wd = (np.asarray(W_out, np.float64)[:, 0]
          - np.asarray(W_out, np.float64)[:, 1])          # [H]
    whd = np.asarray(W_hid, np.float64) @ wd              # [D]
    wfold = (np.asarray(W_emb, np.float64) @ whd).astype(np.float32)  # [E, F]
    bias_e = (np.asarray(b_emb, np.float64) @ whd
              + float(np.asarray(b_hid, np.float64) @ wd)
              + float(b_out[0]) - float(b_out[1]))        # [E]

    xg, mg, tg = node_fea[g], masks[g], t_all[g]
    # chunk each expert group into slices of <=SL nodes; deal S per core
    slices = []
    for e in range(E):
        ix = np.nonzero(tg == e)[0]
        for o in range(0, len(ix), SL):
            slices.append((e, ix[o : o + SL]))
    S = max(1, -(-len(slices) // NCORES))
    assert S * SL <= 96, f"too many slices: {len(slices)}"  # provably <= 3
    W2, c_xT, c_mT, c_wf, c_nb, c_s1, c_s2, c_vd, PKW = _offsets(S)

    in_maps = []
    for c in range(NCORES):
        pk_h = np.zeros((P, PKW), np.float32)
        xT_blk = np.zeros((P, KF, W2), np.float32)
        mT_blk = np.zeros((P, KF, W2), np.float32)
        for s in range(S):
            i = c * S + s
            if i >= len(slices):
                break
            e, ix = slices[i]
            n = len(ix)
            o = s * SL
            xT_blk[:, :, o : o + n] = (
                xg[ix].T.reshape(KF, P, n).transpose(1, 0, 2))
            mT_blk[:, :, o : o + n] = (
                mg[ix].T.reshape(KF, P, n).transpose(1, 0, 2))
            for k in range(KF):
                pk_h[:, c_wf + s * KF + k] = wfold[e, k * P:(k + 1) * P]
            pk_h[o : o + n, c_nb] = -bias_e[e]
            pk_h[o : o + n, c_s1] = 1.0 - 2.0 * lab[ix]
            pk_h[o : o + n, c_s2] = lab[ix]
            pk_h[o : o + n, c_vd] = 1.0
        pk_h[:, c_xT : c_xT + KF * W2] = xT_blk.reshape(P, -1)
        pk_h[:, c_mT : c_mT + KF * W2] = mT_blk.reshape(P, -1)
        in_maps.append({"pk": pk_h})

    nc = _build(S)
    res = run_bass_kernel_spmd(nc, in_maps, core_ids=list(range(NCORES)))
    LAST_RESULTS = res

    probs = np.empty((len(g), 2), np.float32)
    loss = 0.0
    for c in range(NCORES):
        out_c = res.results[c]["out"]
        loss += float(out_c[0, 2])
        for s in range(S):
            i = c * S + s
            if i >= len(slices):
                break
            _, ix = slices[i]
            probs[ix] = out_c[s * SL : s * SL + len(ix), 0:2]
    return probs, np.float32(loss)
